# revision 3
# baseline (speedup 1.0000x reference)
"""Block-sparse linear kernel for Trainium2 (8 NeuronCores, SPMD).

y = W_blocksparse @ x + bias
  x:             [32768, 1024] f32   (128 in-blocks x 256)
  block_weights: [819, 256, 256] f32 (out x in per block)
  bias:          [16384, 1] f32      (64 out-blocks x 256)
  in_idx/out_idx:[819] int32
  y:             [16384, 1024] f32

Sharding: expert-style by out-block. The 64 out-blocks are partitioned into
8 groups of 8 (one per core, balanced by block count). Each core computes
its 8 out-blocks' rows of y over the full batch; outputs are disjoint, so
no collectives. Inputs are packed host-side into per-core fp16 arrays
(weights pre-transposed into lhsT tiles, x tiles pre-gathered per block);
the device program is uniform across cores (SPMD), with zero-weight padding
blocks equalizing per-position block counts.

Device compute: fp16 matmuls (1 cycle/row on the PE) accumulating in fp32
PSUM; bias added during the PSUM->SBUF eviction on the scalar engine.
"""

import functools
import hashlib
import os
import shutil

import numpy as np

NIB = 128      # input blocks
NOBT = 64      # total output blocks
BIN = 256
BOUT = 256
BATCH = 1024
NCORES = 8
NOB = NOBT // NCORES   # out-blocks per core
P = 128

_NEFF_CACHE = os.environ.get(
    "BASS_NEFF_CACHE", os.path.expanduser("~/.cache/bass_neff_cache")
)


def _install_neff_cache():
    """Disk-cache walrus NEFF compiles keyed on the BIR json hash."""
    import concourse.bass2jax as b2j

    if getattr(b2j, "_neff_disk_cache_installed", False):
        return
    orig = b2j.compile_bir_kernel

    def cached(bir_json, tmpdir, neff_name="file.neff"):
        data = bir_json if isinstance(bir_json, bytes) else bir_json.encode()
        key = hashlib.sha256(data).hexdigest()
        cpath = os.path.join(_NEFF_CACHE, key + ".neff")
        if os.path.exists(cpath):
            dst = os.path.join(tmpdir, neff_name)
            shutil.copy(cpath, dst)
            return dst
        out = orig(bir_json, tmpdir, neff_name=neff_name)
        try:
            os.makedirs(_NEFF_CACHE, exist_ok=True)
            tmp = cpath + ".tmp%d" % os.getpid()
            shutil.copy(out, tmp)
            os.replace(tmp, cpath)
        except OSError:
            pass
        return out

    b2j.compile_bir_kernel = cached
    b2j._neff_disk_cache_installed = True


def _plan(in_idx, out_idx):
    """Partition the 64 out-blocks into 8 balanced groups of 8 and compute
    the (cross-core shared) padded per-position block counts."""
    counts = np.bincount(out_idx, minlength=NOBT)
    order = np.argsort(-counts, kind="stable")
    groups = [[] for _ in range(NCORES)]
    tot = [0] * NCORES
    for ob in order:
        cands = [g for g in range(NCORES) if len(groups[g]) < NOB]
        g = min(cands, key=lambda gg: tot[gg])
        groups[g].append(int(ob))
        tot[g] += int(counts[ob])
    for g in range(NCORES):
        groups[g].sort(key=lambda ob: (-counts[ob], ob))
    c = tuple(
        max(1, max(int(counts[groups[g][pos]]) for g in range(NCORES)))
        for pos in range(NOB)
    )
    blocks_by_ob = [np.nonzero(out_idx == ob)[0] for ob in range(NOBT)]
    return groups, c, blocks_by_ob


@functools.lru_cache(maxsize=4)
def _build_program(c):
    """Build + compile the uniform SPMD Tile program for padded counts c."""
    from concourse import bacc, mybir, tile

    f16 = mybir.dt.float16
    f32 = mybir.dt.float32
    NB = sum(c)

    nc = bacc.Bacc("TRN2", target_bir_lowering=False, debug=False,
                   num_devices=NCORES)
    w_ext = nc.dram_tensor("w", [NB, P, 512], f16, kind="ExternalInput").ap()
    xg_ext = nc.dram_tensor("xg", [NB, 2, P, BATCH], f16,
                            kind="ExternalInput").ap()
    b_ext = nc.dram_tensor("bias", [P, 2 * NOB], f32,
                           kind="ExternalInput").ap()
    y_ext = nc.dram_tensor("y", [NOB * BOUT, BATCH], f32,
                           kind="ExternalOutput").ap()

    with tile.TileContext(nc) as tc:
        with tc.tile_pool(name="xp", bufs=16) as xp, \
             tc.tile_pool(name="wp", bufs=16) as wp, \
             tc.tile_pool(name="yp", bufs=6) as yp, \
             tc.tile_pool(name="bp", bufs=1) as bp, \
             tc.tile_pool(name="psp", bufs=8, space="PSUM") as psp:
            bt = bp.tile([P, 2 * NOB], f32, tag="bias", name="bt")
            nc.sync.dma_start(out=bt[:], in_=b_ext[:])
            j0 = 0
            for g in range(NOB):
                ps = [psp.tile([P, 512], f32, tag="ps", name="ps") for _ in range(4)]
                for jj in range(c[g]):
                    j = j0 + jj
                    wt = wp.tile([P, 512], f16, tag="w", name="wt")
                    nc.sync.dma_start(out=wt[:], in_=w_ext[j])
                    for kt in range(2):
                        xt = xp.tile([P, BATCH], f16, tag="x", name="xt")
                        nc.sync.dma_start(out=xt[:], in_=xg_ext[j, kt])
                        for mt in range(2):
                            lhs = wt[:, (kt * 2 + mt) * P:(kt * 2 + mt + 1) * P]
                            for nn in range(2):
                                nc.tensor.matmul(
                                    ps[mt * 2 + nn][:],
                                    lhsT=lhs,
                                    rhs=xt[:, nn * 512:(nn + 1) * 512],
                                    start=(jj == 0 and kt == 0),
                                    stop=(jj == c[g] - 1 and kt == 1),
                                )
                j0 += c[g]
                for mt in range(2):
                    yt = yp.tile([P, BATCH], f32, tag="y", name="yt")
                    for nn in range(2):
                        nc.vector.tensor_scalar_add(
                            out=yt[:, nn * 512:(nn + 1) * 512],
                            in0=ps[mt * 2 + nn][:],
                            scalar1=bt[:, g * 2 + mt:g * 2 + mt + 1],
                        )
                    row = (g * 2 + mt) * P
                    nc.sync.dma_start(out=y_ext[row:row + P, :], in_=yt[:])
    nc.compile()
    return nc


def _pack_inputs(x, block_weights, bias, in_idx, groups, c, blocks_by_ob):
    """Host-side packing into per-core fp16 input arrays."""
    NB = sum(c)
    # lhsT tiles: wpack[n, p, kt, mt, cc] = W[n].T[kt*128+p, mt*128+cc]
    wpack = np.ascontiguousarray(
        block_weights.transpose(0, 2, 1)
        .reshape(-1, 2, P, 2, P)
        .transpose(0, 2, 1, 3, 4)
    ).astype(np.float16).reshape(-1, P, 512)
    x16 = x.astype(np.float16).reshape(NIB, 2, P, BATCH)

    in_maps = []
    for g in range(NCORES):
        w_core = np.zeros((NB, P, 512), np.float16)
        xg_core = np.zeros((NB, 2, P, BATCH), np.float16)
        bias_core = np.zeros((P, 2 * NOB), np.float32)
        j0 = 0
        for pos in range(NOB):
            ob = groups[g][pos]
            blocks = blocks_by_ob[ob]
            nblk = len(blocks)
            if nblk:
                w_core[j0:j0 + nblk] = wpack[blocks]
                xg_core[j0:j0 + nblk] = x16[in_idx[blocks]]
            for mt in range(2):
                bias_core[:, pos * 2 + mt] = bias[ob * BOUT + mt * P:
                                                  ob * BOUT + (mt + 1) * P, 0]
            j0 += c[pos]
        in_maps.append({"w": w_core, "xg": xg_core, "bias": bias_core})
    return in_maps


# Exposed for the test harness: last-built program + inputs for re-timing.
_last = {}


def kernel(x, block_weights, bias, in_idx, out_idx):
    _install_neff_cache()
    from concourse.bass_utils import run_bass_kernel_spmd

    x = np.asarray(x, dtype=np.float32)
    block_weights = np.asarray(block_weights, dtype=np.float32)
    bias = np.asarray(bias, dtype=np.float32)
    in_idx = np.asarray(in_idx, dtype=np.int64)
    out_idx = np.asarray(out_idx, dtype=np.int64)

    groups, c, blocks_by_ob = _plan(in_idx, out_idx)
    nc = _build_program(c)
    in_maps = _pack_inputs(x, block_weights, bias, in_idx, groups, c,
                           blocks_by_ob)

    res = run_bass_kernel_spmd(nc, in_maps, core_ids=list(range(NCORES)))

    y = np.empty((NOBT * BOUT, BATCH), np.float32)
    for g in range(NCORES):
        yc = res.results[g]["y"]
        for pos in range(NOB):
            ob = groups[g][pos]
            y[ob * BOUT:(ob + 1) * BOUT, :] = yc[pos * BOUT:(pos + 1) * BOUT, :]

    _last.update(nc=nc, in_maps=in_maps, groups=groups, c=c)
    return y


# revision 7
# speedup vs baseline: 278.4889x; 278.4889x over previous
"""Block-sparse linear kernel for Trainium2 (8 NeuronCores, SPMD).

y = W_blocksparse @ x + bias
  x:             [32768, 1024] f32   (128 in-blocks x 256)
  block_weights: [819, 256, 256] f32 (out x in per block)
  bias:          [16384, 1] f32      (64 out-blocks x 256)
  in_idx/out_idx:[819] int32
  y:             [16384, 1024] f32

Sharding: expert-style by out-block. The 64 out-blocks are partitioned into
8 groups of 8 (one per core, balanced by block count). Each core computes
its 8 out-blocks' rows of y over the full batch; outputs are disjoint, so
no collectives. Inputs are packed host-side into per-core fp16 arrays
(weights pre-transposed into lhsT tiles, x tiles pre-gathered per block);
the device program is uniform across cores (SPMD), with zero-weight padding
blocks equalizing per-position block counts.

Device compute: fp16 matmuls (1 cycle/row on the PE) accumulating in fp32
PSUM; bias added during the PSUM->SBUF eviction on the scalar engine.
"""

import functools
import hashlib
import os
import shutil

import numpy as np

NIB = 128      # input blocks
NOBT = 64      # total output blocks
BIN = 256
BOUT = 256
BATCH = 1024
NCORES = 8
NOB = NOBT // NCORES   # out-blocks per core
P = 128

_NEFF_CACHE = os.environ.get(
    "BASS_NEFF_CACHE", os.path.expanduser("~/.cache/bass_neff_cache")
)


def _install_neff_cache():
    """Disk-cache walrus NEFF compiles keyed on the BIR json hash."""
    import concourse.bass2jax as b2j

    if getattr(b2j, "_neff_disk_cache_installed", False):
        return
    orig = b2j.compile_bir_kernel

    def cached(bir_json, tmpdir, neff_name="file.neff"):
        data = bir_json if isinstance(bir_json, bytes) else bir_json.encode()
        key = hashlib.sha256(data).hexdigest()
        cpath = os.path.join(_NEFF_CACHE, key + ".neff")
        if os.path.exists(cpath):
            dst = os.path.join(tmpdir, neff_name)
            shutil.copy(cpath, dst)
            return dst
        out = orig(bir_json, tmpdir, neff_name=neff_name)
        try:
            os.makedirs(_NEFF_CACHE, exist_ok=True)
            tmp = cpath + ".tmp%d" % os.getpid()
            shutil.copy(out, tmp)
            os.replace(tmp, cpath)
        except OSError:
            pass
        return out

    b2j.compile_bir_kernel = cached
    b2j._neff_disk_cache_installed = True


def _plan(in_idx, out_idx):
    """Partition the 64 out-blocks into 8 balanced groups of 8 and compute
    the (cross-core shared) padded per-position block counts."""
    counts = np.bincount(out_idx, minlength=NOBT)
    order = np.argsort(-counts, kind="stable")
    groups = [[] for _ in range(NCORES)]
    tot = [0] * NCORES
    for ob in order:
        cands = [g for g in range(NCORES) if len(groups[g]) < NOB]
        g = min(cands, key=lambda gg: tot[gg])
        groups[g].append(int(ob))
        tot[g] += int(counts[ob])
    for g in range(NCORES):
        groups[g].sort(key=lambda ob: (-counts[ob], ob))
    c = tuple(
        max(1, max(int(counts[groups[g][pos]]) for g in range(NCORES)))
        for pos in range(NOB)
    )
    blocks_by_ob = [np.nonzero(out_idx == ob)[0] for ob in range(NOBT)]
    return groups, c, blocks_by_ob


@functools.lru_cache(maxsize=8)
def _build_program(c, iters=1):
    """Build + compile the uniform SPMD Tile program for padded counts c.

    iters > 1 wraps the whole body in an on-device For_i loop repeating the
    identical computation — used only for timing (amortizes dispatch RPC
    overhead into a measurable on-device duration).
    """
    import contextlib

    from concourse import bacc, mybir, tile

    f16 = mybir.dt.float16
    f32 = mybir.dt.float32
    NB = sum(c)

    nc = bacc.Bacc("TRN2", target_bir_lowering=False, debug=False,
                   num_devices=NCORES)
    w_ext = nc.dram_tensor("w", [NB, P, 512], f16, kind="ExternalInput").ap()
    xg_ext = nc.dram_tensor("xg", [NB, 2, P, BATCH], f16,
                            kind="ExternalInput").ap()
    b_ext = nc.dram_tensor("bias", [P, 2 * NOB], f32,
                           kind="ExternalInput").ap()
    y_ext = nc.dram_tensor("y", [NOB * BOUT, BATCH], f32,
                           kind="ExternalOutput").ap()

    with tile.TileContext(nc) as tc:
        with tc.tile_pool(name="xp", bufs=16) as xp, \
             tc.tile_pool(name="wp", bufs=16) as wp, \
             tc.tile_pool(name="yp", bufs=6) as yp, \
             tc.tile_pool(name="bp", bufs=1) as bp, \
             tc.tile_pool(name="psp", bufs=8, space="PSUM") as psp:
            bt = bp.tile([P, 2 * NOB], f32, tag="bias", name="bt")
            nc.sync.dma_start(out=bt[:], in_=b_ext[:])
            loop = (
                tc.For_i(0, iters, 1,
                         hint_engines=(mybir.EngineType.PE,
                                       mybir.EngineType.SP,
                                       mybir.EngineType.DVE))
                if iters > 1 else contextlib.nullcontext()
            )
            with loop:
                _emit_body(nc, tc, c, w_ext, xg_ext, y_ext, bt,
                           xp, wp, yp, psp, f16, f32)
    nc.compile()
    return nc


def _emit_body(nc, tc, c, w_ext, xg_ext, y_ext, bt, xp, wp, yp, psp, f16, f32):
    j0 = 0
    for g in range(NOB):
        ps = [psp.tile([P, 512], f32, tag="ps", name="ps") for _ in range(4)]
        for jj in range(c[g]):
            j = j0 + jj
            wt = wp.tile([P, 512], f16, tag="w", name="wt")
            nc.sync.dma_start(out=wt[:], in_=w_ext[j])
            for kt in range(2):
                xt = xp.tile([P, BATCH], f16, tag="x", name="xt")
                nc.sync.dma_start(out=xt[:], in_=xg_ext[j, kt])
                for mt in range(2):
                    lhs = wt[:, (kt * 2 + mt) * P:(kt * 2 + mt + 1) * P]
                    for nn in range(2):
                        nc.tensor.matmul(
                            ps[mt * 2 + nn][:],
                            lhsT=lhs,
                            rhs=xt[:, nn * 512:(nn + 1) * 512],
                            start=(jj == 0 and kt == 0),
                            stop=(jj == c[g] - 1 and kt == 1),
                        )
        j0 += c[g]
        for mt in range(2):
            yt = yp.tile([P, BATCH], f32, tag="y", name="yt")
            for nn in range(2):
                nc.vector.tensor_scalar_add(
                    out=yt[:, nn * 512:(nn + 1) * 512],
                    in0=ps[mt * 2 + nn][:],
                    scalar1=bt[:, g * 2 + mt:g * 2 + mt + 1],
                )
            row = (g * 2 + mt) * P
            nc.sync.dma_start(out=y_ext[row:row + P, :], in_=yt[:])


def _pack_inputs(x, block_weights, bias, in_idx, groups, c, blocks_by_ob):
    """Host-side packing into per-core fp16 input arrays."""
    NB = sum(c)
    # lhsT tiles: wpack[n, p, kt, mt, cc] = W[n].T[kt*128+p, mt*128+cc]
    wpack = np.ascontiguousarray(
        block_weights.transpose(0, 2, 1)
        .reshape(-1, 2, P, 2, P)
        .transpose(0, 2, 1, 3, 4)
    ).astype(np.float16).reshape(-1, P, 512)
    x16 = x.astype(np.float16).reshape(NIB, 2, P, BATCH)

    in_maps = []
    for g in range(NCORES):
        w_core = np.zeros((NB, P, 512), np.float16)
        xg_core = np.zeros((NB, 2, P, BATCH), np.float16)
        bias_core = np.zeros((P, 2 * NOB), np.float32)
        j0 = 0
        for pos in range(NOB):
            ob = groups[g][pos]
            blocks = blocks_by_ob[ob]
            nblk = len(blocks)
            if nblk:
                w_core[j0:j0 + nblk] = wpack[blocks]
                xg_core[j0:j0 + nblk] = x16[in_idx[blocks]]
            for mt in range(2):
                bias_core[:, pos * 2 + mt] = bias[ob * BOUT + mt * P:
                                                  ob * BOUT + (mt + 1) * P, 0]
            j0 += c[pos]
        in_maps.append({"w": w_core, "xg": xg_core, "bias": bias_core})
    return in_maps


# Exposed for the test harness: last-built program + inputs for re-timing.
_last = {}


def kernel(x, block_weights, bias, in_idx, out_idx):
    _install_neff_cache()
    from concourse.bass_utils import run_bass_kernel_spmd

    x = np.asarray(x, dtype=np.float32)
    block_weights = np.asarray(block_weights, dtype=np.float32)
    bias = np.asarray(bias, dtype=np.float32)
    in_idx = np.asarray(in_idx, dtype=np.int64)
    out_idx = np.asarray(out_idx, dtype=np.int64)

    groups, c, blocks_by_ob = _plan(in_idx, out_idx)
    nc = _build_program(c)
    in_maps = _pack_inputs(x, block_weights, bias, in_idx, groups, c,
                           blocks_by_ob)

    res = run_bass_kernel_spmd(nc, in_maps, core_ids=list(range(NCORES)))

    y = np.empty((NOBT * BOUT, BATCH), np.float32)
    for g in range(NCORES):
        yc = res.results[g]["y"]
        for pos in range(NOB):
            ob = groups[g][pos]
            y[ob * BOUT:(ob + 1) * BOUT, :] = yc[pos * BOUT:(pos + 1) * BOUT, :]

    _last.update(nc=nc, in_maps=in_maps, groups=groups, c=c)
    return y
